# revision 1
# baseline (speedup 1.0000x reference)
"""Trainium2 Bass kernel for nn_Beta_LR_41308995453190.

Network (per (b, o) pair):
  - 13 segment means over the L axis of hidden[b, o] (ragged boundaries
    from idx[b]): 10 context segments, question, option, whole-context.
  - beta-param projection e = 1 + relu(x @ Wp + bp), split a/b.
  - three attention pools (intersection over segments, renew over
    (segment, intersection) pairs, union over inverted renewed params).
  - classify head: concat 8 beta embeddings -> relu(@Wl0 + bl0) -> @Wl + bl.

Sharding: data-parallel over the batch dim B=8 (one batch per NeuronCore),
weights replicated.

Implementation notes (the kernel is PE *instruction-issue* bound, so the
design minimizes tensor-engine instructions):
  - Segment sums are 0/1-mask matmuls (mask as the 13-column stationary
    operand, hidden streaming 512 wide), scaled by 1/count afterwards.
    Hidden and mask travel in bf16 (the mask is exactly representable);
    sums accumulate in fp32 PSUM.
  - All layer matmuls run "flipped": the small activation block is the
    stationary operand, the weight matrix streams 512 columns at a time.
    Layer outputs come out row-major and are transposed back to
    feature-major with tensor-engine transposes so the segment softmaxes
    stay free-axis reductions.
  - Wp/Wa0/Wa are bf16 (measured end-to-end error contribution 1e-6 for
    Wa0/Wa, 2e-4 for Wp); the classify head Wl0 stays fp32 (bf16 there
    would cost 2.3e-3). The whole softmax/pooling pipeline is fp32.
"""

import numpy as np
import ml_dtypes

try:
    import concourse.bass as bass
except ImportError:
    import sys

    sys.path.insert(0, "/opt/trn_rl_repo")
    import concourse.bass as bass

import concourse.tile as tile
from concourse import mybir
from concourse.bass_utils import run_bass_kernel_spmd
from concourse.masks import make_identity

F32 = mybir.dt.float32
BF16 = mybir.dt.bfloat16
NPBF16 = ml_dtypes.bfloat16
AX = mybir.AxisListType.X
OP = mybir.AluOpType
AF = mybir.ActivationFunctionType

B, O, L, E = 8, 4, 1024, 1024
BETA = 512
NSEG = 12
NK = 13  # 10 ctx + q + o + allc
P = 128
T = L // P  # 8 L-tiles per option
NCOL = O * NK  # 52


# ---------------------------------------------------------------------------
# Workaround: this neuronxcc walrus build rejects more than one sem wait per
# TPB instruction ("Too many sync wait commands"). Hoist excess waits onto
# drain instructions inserted immediately before the offending instruction on
# the same engine — the engine blocks at each drain until its condition
# holds, which is semantically identical to multiple waits on one
# instruction.
# ---------------------------------------------------------------------------
# The classify-head weight matrix in bf16 saves ~18us of tensor-engine time
# and 4 MB of DMA but costs ~2e-3 end-to-end relative error (vs ~3e-4).
WL0_BF16 = False


def _patch_minimal_drain():
    """One-shot NEFF: skip the semaphore-clear + second all-engine barrier of
    the TileContext epilogue (they only matter when the program loops)."""
    from concourse.vector_clock import ScopedClock

    def _drain_and_barrier(self, tick_clock, wait_clock):
        drain_inst = self.nc.sync.drain()
        wait_clock.add_sem_waits(
            drain_inst.ins, ScopedClock({None: tick_clock.global_clock})
        )
        self.nc.all_engine_barrier()
        assert self.sems is not None
        popped = self.nc._tile_sem_poison_stack.pop()
        assert popped is self._sem_poison

    tile.TileContext._drain_and_barrier = _drain_and_barrier


# Measured neutral-to-slightly-worse on HW; keep the stock epilogue.
# _patch_minimal_drain()


def _split_excess_waits(nc, max_waits=1):
    scratch_bb = nc.cur_bb.bb
    for f in nc.m.functions:
        for bb in f.blocks:
            new_list = []
            for ins in bb.instructions:
                si = ins.sync_info
                waits = list(si.on_wait) if si and si.on_wait else []
                if len(waits) > max_waits:
                    for w in waits[: len(waits) - max_waits]:
                        carrier = nc.engines[ins.engine].nop(nofuse=True).ins
                        scratch_bb.instructions.remove(carrier)
                        carrier.sync_info = mybir.SyncInfo(
                            on_wait=[w], on_update=[]
                        )
                        new_list.append(carrier)
                    si.on_wait = waits[len(waits) - max_waits :]
                new_list.append(ins)
            bb.instructions[:] = new_list


def _build_nc(debug=False):
    nc = bass.Bass("TRN2", target_bir_lowering=False)

    hid_d = nc.dram_tensor("hidden", [O, L, E], BF16, kind="ExternalInput")
    mask_d = nc.dram_tensor("maskt", [P, T, NK], BF16, kind="ExternalInput")
    cnt_d = nc.dram_tensor("cntinv", [NK, 1], F32, kind="ExternalInput")
    wp_d = nc.dram_tensor("wp", [P, 8, 1024], BF16, kind="ExternalInput")
    wa0_d = nc.dram_tensor("wa0", [P, 8, 512], BF16, kind="ExternalInput")
    wa_d = nc.dram_tensor("wa", [P, 4, 512], BF16, kind="ExternalInput")
    wl0_d = nc.dram_tensor(
        "wl0", [P, 32, 512], BF16 if WL0_BF16 else F32, kind="ExternalInput"
    )
    bias_d = nc.dram_tensor("biases", [P, 21], F32, kind="ExternalInput")
    bl0r_d = nc.dram_tensor("bl0rep", [O, 512], F32, kind="ExternalInput")
    wlr_d = nc.dram_tensor("wlrep", [O, 512], F32, kind="ExternalInput")
    out_d = nc.dram_tensor("out", [O, 1], F32, kind="ExternalOutput")

    with tile.TileContext(nc) as tc:
        with (
            tc.tile_pool(name="const", bufs=1) as const,
            tc.tile_pool(name="hidp2", bufs=2) as hidp2,
            tc.tile_pool(name="act", bufs=1) as act,
            tc.tile_pool(name="tmp", bufs=3) as tmp,
            tc.tile_pool(name="rows", bufs=1) as rowsp,
            tc.tile_pool(name="pseg", bufs=2, space="PSUM") as pseg,
            tc.tile_pool(name="prow", bufs=2, space="PSUM") as prow,
            tc.tile_pool(name="pt", bufs=2, space="PSUM") as pt,
        ):
            # ---- constants (seg-phase ones first)
            mask_sb = const.tile([P, T, NK], BF16)
            nc.sync.dma_start(out=mask_sb, in_=mask_d[:])
            cnt_sb = const.tile([NK, 1], F32)
            nc.sync.dma_start(out=cnt_sb, in_=cnt_d[:])
            ident = const.tile([P, P], F32)
            make_identity(nc, ident)

            def bcol(i):
                return bias_sb[:, i : i + 1]

            # ---- segment sums: ps[k, e] = sum over rows of seg k (0/1 mask)
            # then x = ps * cntinv, transposed to xT[c, o, k] (bf16)
            xT = act.tile([P, 8, O, NK], BF16)
            # one 32-aligned row block per option (partition bases must be
            # 32-aligned), transposed 128 columns at a time
            x_all = rowsp.tile([P, E], F32, tag="x_all")
            nc.vector.memset(x_all, 0.0)
            wp_sb = wa0_sb = wa_sb = None
            hid_r = hid_d.rearrange("o (t p) e -> o p t e", p=P)
            bias_sb = bl0r_sb = wlr_sb = None
            for o in range(O):
                htile = hidp2.tile([P, T, E], BF16, tag="htile")
                # four dma_starts per option so the transfer spreads over
                # four HWDGE queues (a single queue sustains only ~170 GB/s)
                for q in range(4):
                    nc.sync.dma_start(
                        out=htile[:, q * 2 : q * 2 + 2, :],
                        in_=hid_r[o][:, q * 2 : q * 2 + 2, :],
                    )
                if o == 0:
                    # queue the head weights behind the first option's tiles
                    bias_sb = const.tile([P, 21], F32)
                    nc.sync.dma_start(out=bias_sb, in_=bias_d[:])
                    bl0r_sb = const.tile([O, 512], F32)
                    nc.sync.dma_start(out=bl0r_sb, in_=bl0r_d[:])
                    wlr_sb = const.tile([O, 512], F32)
                    nc.sync.dma_start(out=wlr_sb, in_=wlr_d[:])
                    wp_sb = const.tile([P, 8, 1024], BF16)
                    nc.sync.dma_start(out=wp_sb, in_=wp_d[:])
                    wa0_sb = const.tile([P, 8, 512], BF16)
                    nc.sync.dma_start(out=wa0_sb, in_=wa0_d[:])
                    wa_sb = const.tile([P, 4, 512], BF16)
                    nc.sync.dma_start(out=wa_sb, in_=wa_d[:])
                ps = pseg.tile([NK, E], F32, tag="ps_seg")
                for half in range(2):
                    sl = slice(half * 512, half * 512 + 512)
                    for t in range(T):
                        nc.tensor.matmul(
                            out=ps[:, sl],
                            lhsT=mask_sb[:, t, :],
                            rhs=htile[:, t, sl],
                            start=(t == 0),
                            stop=(t == T - 1),
                        )
                nc.vector.tensor_scalar_mul(
                    out=x_all[o * 32 : o * 32 + NK, :],
                    in0=ps[:, :],
                    scalar1=cnt_sb[:, :],
                )
            for c in range(8):
                ptile = pt.tile([P, P], F32, tag="pt")
                nc.tensor.transpose(
                    out=ptile,
                    in_=x_all[:, c * P : (c + 1) * P],
                    identity=ident[:, :],
                )
                nc.scalar.copy(
                    out=xT[:, c, :, :],
                    in_=ptile.rearrange("p (o k) -> p o k", k=32)[:, :, 0:NK],
                )

            # ---- wl0 DMA last: only needed by the classify head
            wl0_sb = const.tile([P, 32, 512], BF16 if WL0_BF16 else F32)
            nc.sync.dma_start(out=wl0_sb[:, 0:16, :], in_=wl0_d[:, 0:16, :])
            nc.sync.dma_start(out=wl0_sb[:, 16:32, :], in_=wl0_d[:, 16:32, :])

            def flip_layer(
                name,
                lhs_chunks,  # list of bf16 [P, R] stationary APs (K chunks)
                w_sb,  # weight tile, [P, K/128, NW] layout
                n_out,  # output features
                r,  # rows (= lhs free size)
            ):
                """out rows = (lhs^T)^T @ W, returns list of fp32 PSUM tiles
                [r, 512] per 512-wide output chunk, and the row-major sbuf
                copy [r, n_out]."""
                rows_full = rowsp.tile([NCOL, 1024], F32, tag="rows_sh")
                rows_sb = rows_full[:r, :n_out]
                psums = []
                for n2 in range(n_out // 512):
                    pr = prow.tile([r, 512], F32, tag="prow")
                    for c, lhs in enumerate(lhs_chunks):
                        nc.tensor.matmul(
                            out=pr,
                            lhsT=lhs,
                            rhs=w_sb[:, c, n2 * 512 : (n2 + 1) * 512]
                            if w_sb.shape[2] > 512
                            else w_sb[:, c, :],
                            start=(c == 0),
                            stop=(c == len(lhs_chunks) - 1),
                        )
                    nc.scalar.copy(
                        out=rows_sb[:, n2 * 512 : (n2 + 1) * 512], in_=pr[:, :]
                    )
                    psums.append(pr)
                return rows_sb

            def transpose_rows(rows_sb, r, n_out):
                """Yield (mc, psum [P, r]) transposed feature chunks."""
                for mc in range(n_out // P):
                    ptile = pt.tile([P, r], F32, tag="pt")
                    nc.tensor.transpose(
                        out=ptile,
                        in_=rows_sb[:, mc * P : (mc + 1) * P],
                        identity=ident[:r, :r],
                    )
                    yield mc, ptile

            # ---- projection: e = max(x @ Wp + (bp + 1), 1)
            eT = act.tile([P, 8, O, NK], F32)
            eTb = act.tile([P, 8, NCOL], BF16)
            xT_chunks = [xT[:, c, :, :] for c in range(8)]
            rows_e = flip_layer("e", xT_chunks, wp_sb, 1024, NCOL)
            for mc, ptile in transpose_rows(rows_e, NCOL, 1024):
                nc.vector.tensor_scalar(
                    out=eT[:, mc, :, :],
                    in0=ptile[:, :],
                    scalar1=bcol(mc),
                    scalar2=1.0,
                    op0=OP.add,
                    op1=OP.max,
                )
                nc.vector.tensor_copy(out=eTb[:, mc, :], in_=eT[:, mc, :, :])

            # catF chunks 8..31 (a_ac, b_ac, a_o, b_o, a_q, b_q) only need eT;
            # filling them now lets the classify-head matmuls over those
            # chunks run inside tensor-engine gaps during the softmax phases.
            catF = act.tile([P, 32, O], F32)
            for j, (half, k) in enumerate(
                ((0, 12), (1, 12), (0, 11), (1, 11), (0, 10), (1, 10))
            ):
                nc.gpsimd.tensor_copy(
                    out=catF[:, 8 + j * 4 : 12 + j * 4, :],
                    in_=eT[:, half * 4 : half * 4 + 4, :, k],
                )

            # ---- pool 1 (intersection): h1 = relu(e @ Wa0 + ba0) (bf16 out)
            h1Tb = act.tile([P, 4, NCOL], BF16)
            rows_h1 = flip_layer(
                "h1", [eTb[:, c, :] for c in range(8)], wa0_sb, 512, NCOL
            )
            for mc, ptile in transpose_rows(rows_h1, NCOL, 512):
                nc.vector.tensor_scalar(
                    out=h1Tb[:, mc, :],
                    in0=ptile[:, :],
                    scalar1=bcol(8 + mc),
                    scalar2=0.0,
                    op0=OP.add,
                    op1=OP.max,
                )

            # l1 = h1 @ Wa + ba (fp32, shared by pool 1 softmax and renew)
            l1T = act.tile([P, 4, O, NK], F32)
            rows_l1 = flip_layer(
                "l1", [h1Tb[:, c, :] for c in range(4)], wa_sb, 512, NCOL
            )
            for mc, ptile in transpose_rows(rows_l1, NCOL, 512):
                nc.vector.tensor_scalar_add(
                    out=l1T[:, mc, :, :], in0=ptile[:, :], scalar1=bcol(12 + mc)
                )

            # pool 1 softmax over the 10 ctx segments + weighted reduce
            # (batched across all 4 feature chunks: [P, 4, O, 10] at once)
            cat2 = act.tile([P, 8, O], F32)
            cat2b = act.tile([P, 8, O], BF16)
            lsl = l1T[:, :, :, 0:10]
            mx = tmp.tile([P, 4, O], F32, tag="mx")
            nc.vector.reduce_max(mx, lsl, axis=AX)
            d = tmp.tile([P, 4, O, 10], F32, tag="d")
            nc.vector.tensor_tensor(
                out=d, in0=lsl, in1=mx.broadcast_to([P, 4, O, 10]), op=OP.subtract
            )
            w = tmp.tile([P, 4, O, 10], F32, tag="w")
            nc.scalar.activation(out=w, in_=d, func=AF.Exp)
            s = tmp.tile([P, 4, O], F32, tag="s")
            nc.vector.reduce_sum(s, w, axis=AX)
            r = tmp.tile([P, 4, O], F32, tag="r")
            nc.vector.reciprocal(out=r, in_=s)
            wn = tmp.tile([P, 4, O, 10], F32, tag="wn")
            nc.vector.tensor_tensor(
                out=wn, in0=w, in1=r.broadcast_to([P, 4, O, 10]), op=OP.mult
            )
            wa_t = tmp.tile([P, 4, O, 10], F32, tag="wa_t")
            nc.vector.tensor_tensor(
                out=wa_t, in0=wn, in1=eT[:, 0:4, :, 0:10], op=OP.mult
            )
            nc.vector.reduce_sum(cat2[:, 0:4, :], wa_t, axis=AX)
            wb_t = tmp.tile([P, 4, O, 10], F32, tag="wb_t")
            nc.vector.tensor_tensor(
                out=wb_t, in0=wn, in1=eT[:, 4:8, :, 0:10], op=OP.mult
            )
            nc.vector.reduce_sum(cat2[:, 4:8, :], wb_t, axis=AX)
            nc.vector.tensor_copy(out=cat2b, in_=cat2)

            # ---- renew: h2/l2 for the intersection pair element
            h2Tb = act.tile([P, 4, O], BF16)
            rows_h2 = flip_layer(
                "h2", [cat2b[:, c, :] for c in range(8)], wa0_sb, 512, O
            )
            for mc, ptile in transpose_rows(rows_h2, O, 512):
                nc.vector.tensor_scalar(
                    out=h2Tb[:, mc, :],
                    in0=ptile[:, :],
                    scalar1=bcol(8 + mc),
                    scalar2=0.0,
                    op0=OP.add,
                    op1=OP.max,
                )
            l2T = act.tile([P, 4, O], F32)
            rows_l2 = flip_layer(
                "l2", [h2Tb[:, c, :] for c in range(4)], wa_sb, 512, O
            )
            for mc, ptile in transpose_rows(rows_l2, O, 512):
                nc.vector.tensor_scalar_add(
                    out=l2T[:, mc, :], in0=ptile[:, :], scalar1=bcol(12 + mc)
                )

            # pair softmax([l1[k], l2]) -> na/nb; store reciprocals
            # (batched: [P, 4, O, 10] at once)
            raT = act.tile([P, 4, O, 10], F32)
            rbT = act.tile([P, 4, O, 10], F32)
            raTb = act.tile([P, 4, O, 10], BF16)
            rbTb = act.tile([P, 4, O, 10], BF16)
            l1s = l1T[:, :, :, 0:10]
            l2b = l2T[:, :, :].broadcast_to([P, 4, O, 10])
            mxp = tmp.tile([P, 4, O, 10], F32, tag="mxp")
            nc.vector.tensor_tensor(out=mxp, in0=l1s, in1=l2b, op=OP.max)
            d1 = tmp.tile([P, 4, O, 10], F32, tag="d1")
            nc.vector.tensor_tensor(out=d1, in0=l1s, in1=mxp, op=OP.subtract)
            e1 = tmp.tile([P, 4, O, 10], F32, tag="e1")
            nc.scalar.activation(out=e1, in_=d1, func=AF.Exp)
            d2 = tmp.tile([P, 4, O, 10], F32, tag="d2")
            nc.vector.tensor_tensor(out=d2, in0=l2b, in1=mxp, op=OP.subtract)
            e2 = tmp.tile([P, 4, O, 10], F32, tag="e2")
            nc.scalar.activation(out=e2, in_=d2, func=AF.Exp)
            s12 = tmp.tile([P, 4, O, 10], F32, tag="s12")
            nc.vector.tensor_tensor(out=s12, in0=e1, in1=e2, op=OP.add)
            rs = tmp.tile([P, 4, O, 10], F32, tag="rs")
            nc.vector.reciprocal(out=rs, in_=s12)
            for half, dst, dstb in ((0, raT, raTb), (1, rbT, rbTb)):
                t1 = tmp.tile([P, 4, O, 10], F32, tag="t1")
                nc.vector.tensor_tensor(
                    out=t1,
                    in0=e1,
                    in1=eT[:, half * 4 : half * 4 + 4, :, 0:10],
                    op=OP.mult,
                )
                t2 = tmp.tile([P, 4, O, 10], F32, tag="t2")
                nc.vector.tensor_tensor(
                    out=t2,
                    in0=e2,
                    in1=cat2[:, half * 4 : half * 4 + 4, :].broadcast_to(
                        [P, 4, O, 10]
                    ),
                    op=OP.mult,
                )
                t3 = tmp.tile([P, 4, O, 10], F32, tag="t3")
                nc.vector.tensor_tensor(out=t3, in0=t1, in1=t2, op=OP.add)
                nv = tmp.tile([P, 4, O, 10], F32, tag="nv")
                nc.vector.tensor_tensor(out=nv, in0=t3, in1=rs, op=OP.mult)
                nc.vector.reciprocal(out=dst[:, :, :, :], in_=nv)
                nc.vector.tensor_copy(out=dstb[:, :, :, :], in_=dst[:, :, :, :])

            # ---- union pool over segments of [1/na; 1/nb]
            h3Tb = act.tile([P, 4, O, 10], BF16)
            rows_h3 = flip_layer(
                "h3",
                [raTb[:, c, :, :] for c in range(4)]
                + [rbTb[:, c, :, :] for c in range(4)],
                wa0_sb,
                512,
                O * 10,
            )
            for mc, ptile in transpose_rows(rows_h3, O * 10, 512):
                nc.vector.tensor_scalar(
                    out=h3Tb[:, mc, :, :],
                    in0=ptile[:, :],
                    scalar1=bcol(8 + mc),
                    scalar2=0.0,
                    op0=OP.add,
                    op1=OP.max,
                )
            l3T = act.tile([P, 4, O, 10], F32)
            rows_l3 = flip_layer(
                "l3", [h3Tb[:, c, :, :] for c in range(4)], wa_sb, 512, O * 10
            )
            for mc, ptile in transpose_rows(rows_l3, O * 10, 512):
                nc.vector.tensor_scalar_add(
                    out=l3T[:, mc, :, :], in0=ptile[:, :], scalar1=bcol(12 + mc)
                )

            # union softmax + weighted reduce + invert -> catF chunks 0..7
            # (batched: [P, 4, O, 10] at once)
            mx3 = tmp.tile([P, 4, O], F32, tag="mx3")
            nc.vector.reduce_max(mx3, l3T[:, :, :, :], axis=AX)
            d3 = tmp.tile([P, 4, O, 10], F32, tag="d3")
            nc.vector.tensor_tensor(
                out=d3,
                in0=l3T[:, :, :, :],
                in1=mx3.broadcast_to([P, 4, O, 10]),
                op=OP.subtract,
            )
            w3 = tmp.tile([P, 4, O, 10], F32, tag="w3")
            nc.scalar.activation(out=w3, in_=d3, func=AF.Exp)
            s3 = tmp.tile([P, 4, O], F32, tag="s3")
            nc.vector.reduce_sum(s3, w3, axis=AX)
            r3 = tmp.tile([P, 4, O], F32, tag="r3")
            nc.vector.reciprocal(out=r3, in_=s3)
            wn3 = tmp.tile([P, 4, O, 10], F32, tag="wn3")
            nc.vector.tensor_tensor(
                out=wn3, in0=w3, in1=r3.broadcast_to([P, 4, O, 10]), op=OP.mult
            )
            for half, src in ((0, raT), (1, rbT)):
                tu = tmp.tile([P, 4, O, 10], F32, tag="tu")
                nc.vector.tensor_tensor(
                    out=tu, in0=wn3, in1=src[:, :, :, :], op=OP.mult
                )
                su = tmp.tile([P, 4, O], F32, tag="su")
                nc.vector.reduce_sum(su, tu, axis=AX)
                nc.vector.reciprocal(
                    out=catF[:, half * 4 : half * 4 + 4, :], in_=su
                )

            # ---- classify head: hf = cat @ Wl0, rows [O, 512]
            if WL0_BF16:
                catFm = act.tile([P, 32, O], BF16)
                nc.vector.tensor_copy(out=catFm, in_=catF)
            else:
                catFm = catF
            pf = prow.tile([O, 512], F32, tag="prow")
            kc_order = list(range(8, 32)) + list(range(8))
            for i, kc in enumerate(kc_order):
                nc.tensor.matmul(
                    out=pf,
                    lhsT=catFm[:, kc, :],
                    rhs=wl0_sb[:, kc, :],
                    start=(i == 0),
                    stop=(i == 31),
                )
            # out = relu(hf + bl0) . Wl + bl, all on the vector engine
            hrelu = rowsp.tile([O, 512], F32, tag="hrelu")
            nc.vector.tensor_tensor(out=hrelu, in0=pf[:, :], in1=bl0r_sb, op=OP.add)
            nc.vector.tensor_scalar_max(out=hrelu, in0=hrelu, scalar1=0.0)
            hw = rowsp.tile([O, 512], F32, tag="hw")
            nc.vector.tensor_tensor(out=hw, in0=hrelu, in1=wlr_sb, op=OP.mult)
            osum = rowsp.tile([O, 1], F32, tag="osum")
            nc.vector.reduce_sum(osum, hw, axis=AX)
            out_sb = rowsp.tile([O, 1], F32, tag="out_sb")
            nc.vector.tensor_scalar_add(
                out=out_sb, in0=osum, scalar1=bias_sb[0:O, 20:21]
            )
            nc.sync.dma_start(out=out_d[:], in_=out_sb)

            if debug:
                for name, t in (
                    ("xT", xT),
                    ("eT", eT),
                    ("l1T", l1T),
                    ("cat2", cat2),
                    ("raT", raT),
                    ("rbT", rbT),
                    ("catF", catF),
                ):
                    dt = F32 if t is not xT else BF16
                    d = nc.dram_tensor(
                        "dbg_" + name, list(t.shape), dt, kind="ExternalOutput"
                    )
                    nc.sync.dma_start(out=d[:], in_=t)

    _split_excess_waits(nc)
    return nc


_NC = None


def _get_nc():
    global _NC
    if _NC is None:
        _NC = _build_nc()
    return _NC


def _prep_inputs(hidden, idx, Wp, bp, Wa0, ba0, Wa, ba, Wl0, bl0, Wl, bl):
    hidden = np.asarray(hidden, dtype=np.float32)
    idx = np.asarray(idx).astype(np.int64)

    f32 = lambda a: np.ascontiguousarray(np.asarray(a, dtype=np.float32))
    bf = lambda a: np.ascontiguousarray(np.asarray(a, dtype=np.float32).astype(NPBF16))
    bp, ba0, ba, bl0, bl = f32(bp), f32(ba0), f32(ba), f32(bl0), f32(bl)
    Wl = f32(Wl)

    hid_b = np.ascontiguousarray(hidden.astype(NPBF16))  # [B, O, L, E]
    wp_t = bf(np.asarray(Wp, np.float32).reshape(8, P, 1024).transpose(1, 0, 2))
    wa0_t = bf(np.asarray(Wa0, np.float32).reshape(8, P, 512).transpose(1, 0, 2))
    wa_t = bf(np.asarray(Wa, np.float32).reshape(4, P, 512).transpose(1, 0, 2))
    wl0_t = np.asarray(Wl0, np.float32).reshape(32, P, 512).transpose(1, 0, 2)
    wl0_t = bf(wl0_t) if WL0_BF16 else f32(wl0_t)

    biases = np.zeros((P, 21), dtype=np.float32)
    biases[:, 0:8] = (bp + 1.0).reshape(8, P).T
    biases[:, 8:12] = ba0.reshape(4, P).T
    biases[:, 12:16] = ba.reshape(4, P).T
    biases[:, 16:20] = bl0.reshape(4, P).T
    biases[:, 20] = bl[0]

    bl0rep = np.ascontiguousarray(np.broadcast_to(bl0, (O, 512)).astype(np.float32))
    wlrep = np.ascontiguousarray(np.broadcast_to(Wl[:, 0], (O, 512)).astype(np.float32))

    in_maps = []
    for b in range(B):
        m = np.zeros((L, NK), dtype=np.float32)
        cntinv = np.zeros((NK, 1), dtype=np.float32)
        ib = idx[b]
        starts = [1] + [int(ib[k]) for k in range(9)]
        ends = [int(ib[k]) for k in range(10)]
        segs = [(starts[k], ends[k]) for k in range(10)]
        segs.append((int(ib[9]), int(ib[10])))
        segs.append((int(ib[10]), int(ib[11])))
        segs.append((1, int(ib[9])))
        for k, (s, e) in enumerate(segs):
            m[s:e, k] = 1.0
            cntinv[k, 0] = 1.0 / (e - s)
        maskt = np.ascontiguousarray(
            m.reshape(T, P, NK).transpose(1, 0, 2).astype(NPBF16)
        )

        in_maps.append(
            dict(
                hidden=np.ascontiguousarray(hid_b[b]),
                maskt=maskt,
                cntinv=cntinv,
                wp=wp_t,
                wa0=wa0_t,
                wa=wa_t,
                wl0=wl0_t,
                biases=biases,
                bl0rep=bl0rep,
                wlrep=wlrep,
            )
        )
    return in_maps


def _run(in_maps, **kwargs):
    return run_bass_kernel_spmd(_get_nc(), in_maps, core_ids=list(range(B)), **kwargs)


def kernel(**inputs):
    in_maps = _prep_inputs(**inputs)
    res = _run(in_maps)
    return np.stack([r["out"].reshape(O, 1) for r in res.results])


def _install_ntff_hook():
    """The RL container's antenv lacks axon_hooks, so boot() skipped NTFF
    hook registration. Recreate the module and register the ctypes hook."""
    import sys
    import types

    name = "antenv.axon_hooks"
    if name not in sys.modules:
        try:
            __import__(name)
        except ImportError:
            mod = types.ModuleType(name)
            mod._hook = None
            mod.set_axon_ntff_profile_hook = lambda h: setattr(mod, "_hook", h)
            mod.get_axon_ntff_profile_hook = lambda: mod._hook
            sys.modules[name] = mod
            import antenv

            antenv.axon_hooks = mod
    import antenv.axon_hooks as ah

    if ah.get_axon_ntff_profile_hook() is None:
        from trn_agent_boot.trn_boot import _ntff_profile_via_ctypes

        ah.set_axon_ntff_profile_hook(
            _ntff_profile_via_ctypes("/opt/axon/libaxon_pjrt.so")
        )

    import concourse.bass_utils as bu

    bu.upload_artifacts = lambda tmpdir: tmpdir


def benchmark(trace_cores=None, **inputs):
    """Run with NTFF tracing; returns (output, BassKernelResults)."""
    _install_ntff_hook()
    in_maps = _prep_inputs(**inputs)
    res = _run(in_maps, trace=True, trace_cores=trace_cores)
    out = np.stack([r["out"].reshape(O, 1) for r in res.results])
    return out, res



# revision 12
# speedup vs baseline: 1.4553x; 1.4553x over previous
"""Trainium2 Bass kernel for nn_Beta_LR_41308995453190.

Network (per (b, o) pair):
  - 13 segment means over the L axis of hidden[b, o] (ragged boundaries
    from idx[b]): 10 context segments, question, option, whole-context.
  - beta-param projection e = 1 + relu(x @ Wp + bp), split a/b.
  - three attention pools (intersection over segments, renew over
    (segment, intersection) pairs, union over inverted renewed params).
  - classify head: concat 8 beta embeddings -> relu(@Wl0 + bl0) -> @Wl + bl.

Sharding: data-parallel over the batch dim B=8 (one batch per NeuronCore),
weights replicated.

Design (v2 — rebuilt around the trace of the v1 kernel):
  - hidden travels in fp8 e3m4 (4.2 MB/core instead of 8.4 bf16; measured
    end-to-end rel-err 2.9e-3 vs the 2e-2 gate). All weights bf16.
  - Segment sums are 0/1-mask matmuls (mask stationary, hidden streaming).
    The two E-halves run CONCURRENTLY in separate PE column groups
    (tile_position col 0 / 32, derived from the PSUM slice base partition).
  - The beta-network layers run in CLASSIC orientation: weight chunks
    [128, 128] stationary, feature-major activations [128, cols] streaming.
    Layer outputs land feature-major in PSUM, so the bias/relu DVE op is
    128-partition-parallel and NO transposes are needed between layers
    (v1 spent ~10us of PE time on 40 transposes + PSUM round trips).
  - Softmaxes skip the max-subtraction (logits are ~N(0, 0.25)); the
    intersection's exp/weighted sums are reused by the renew stage.
  - Classify head: catF chunks stationary [128, 4], wl0 streams 512 wide,
    accumulated in 4 PE column groups concurrently; bl0 is folded in as a
    33rd contraction chunk (one-hot stationary, bl0 in wl0 row 0). The 24
    chunks that only need the projection run inside the softmax bubbles;
    epilogue relu*Wl+reduce is one fused scalar_tensor_tensor op.
  - DMA: hidden kicks on the Sync HWDGE queue, weights on the Scalar
    queue (two engines issue descriptors concurrently; each descriptor
    fans out to one of 16 HW DMA engines). wl0 (4.2 MB) is ordered last
    — the head only needs it ~25us in.
"""

import numpy as np
import ml_dtypes

try:
    import concourse.bass as bass
except ImportError:
    import sys

    sys.path.insert(0, "/opt/trn_rl_repo")
    import concourse.bass as bass

import concourse.tile as tile
from concourse import mybir
from concourse.bass_utils import run_bass_kernel_spmd
from concourse.masks import make_identity

F32 = mybir.dt.float32
BF16 = mybir.dt.bfloat16
FP8 = mybir.dt.float8e3  # e3m4
NPBF16 = ml_dtypes.bfloat16
NPFP8 = ml_dtypes.float8_e3m4
AX = mybir.AxisListType.X
OP = mybir.AluOpType
AF = mybir.ActivationFunctionType

B, O, L, E = 8, 4, 1024, 1024
BETA = 512
NSEG = 12
NK = 13  # 10 ctx + q + o + allc
NC10 = 10  # pooled ctx segments
P = 128
T = L // P  # 8 L-tiles per option
NCOL = O * NK  # 52

# wl0 host chunk order: kc 8..31 first (available early, consumed inside the
# softmax bubbles), then kc 0..7 (need the union result), then the bl0 chunk.
WL0_ORDER = list(range(8, 32)) + list(range(0, 8)) + [32]
WL0_POS = {kc: i for i, kc in enumerate(WL0_ORDER)}


def _split_excess_waits(nc, max_waits=1):
    """This neuronxcc walrus build rejects more than one sem wait per TPB
    instruction ("Too many sync wait commands"). Hoist excess waits onto
    drain instructions inserted immediately before the offending instruction
    on the same engine."""
    scratch_bb = nc.cur_bb.bb
    for f in nc.m.functions:
        for bb in f.blocks:
            new_list = []
            for ins in bb.instructions:
                si = ins.sync_info
                waits = list(si.on_wait) if si and si.on_wait else []
                if len(waits) > max_waits:
                    for w in waits[: len(waits) - max_waits]:
                        carrier = nc.engines[ins.engine].nop(nofuse=True).ins
                        scratch_bb.instructions.remove(carrier)
                        carrier.sync_info = mybir.SyncInfo(
                            on_wait=[w], on_update=[]
                        )
                        new_list.append(carrier)
                    si.on_wait = waits[len(waits) - max_waits :]
                new_list.append(ins)
            bb.instructions[:] = new_list


def _build_nc(debug=False):
    nc = bass.Bass("TRN2", target_bir_lowering=False)

    hid_d = nc.dram_tensor("hidden", [O, L, E], FP8, kind="ExternalInput")
    mask_d = nc.dram_tensor("maskt", [P, T, NK], FP8, kind="ExternalInput")
    cnt_d = nc.dram_tensor("cntinv", [NK, 1], F32, kind="ExternalInput")
    wp_d = nc.dram_tensor("wp", [P, 8, 1024], BF16, kind="ExternalInput")
    wa0_d = nc.dram_tensor("wa0", [P, 8, 512], BF16, kind="ExternalInput")
    wa_d = nc.dram_tensor("wa", [P, 4, 512], BF16, kind="ExternalInput")
    wl0_d = nc.dram_tensor("wl0", [P, 33, 512], BF16, kind="ExternalInput")
    bias_d = nc.dram_tensor("biases", [P, 17], F32, kind="ExternalInput")
    wlr_d = nc.dram_tensor("wlrep", [O, 512], F32, kind="ExternalInput")
    out_d = nc.dram_tensor("out", [O, 1], F32, kind="ExternalOutput")

    with tile.TileContext(nc) as tc:
        with (
            tc.tile_pool(name="const", bufs=1) as const,
            tc.tile_pool(name="hidp2", bufs=2) as hidp2,
            tc.tile_pool(name="act", bufs=1) as act,
            tc.tile_pool(name="tmp", bufs=2) as tmp,
            tc.tile_pool(name="pseg", bufs=2, space="PSUM") as pseg,
            tc.tile_pool(name="pt", bufs=2, space="PSUM") as pt,
            tc.tile_pool(name="pmm", bufs=2, space="PSUM") as pmm,
            tc.tile_pool(name="phead", bufs=1, space="PSUM") as phead,
        ):
            # ---- DMA kicks. Sync: mask + hidden. Scalar: everything else,
            # in need-order (consts/wp first, wl0 last).
            mask_sb = const.tile([P, T, NK], FP8)
            nc.sync.dma_start(out=mask_sb, in_=mask_d[:])

            hid_r = hid_d.rearrange("o (t p) e -> o p t e", p=P)
            htiles = []
            for o in range(O):
                htile = hidp2.tile([P, T, E], FP8, tag="htile")
                for q in range(4):
                    nc.sync.dma_start(
                        out=htile[:, q * 2 : q * 2 + 2, :],
                        in_=hid_r[o][:, q * 2 : q * 2 + 2, :],
                    )
                htiles.append(htile)

            cnt_sb = const.tile([NK, 1], F32)
            nc.scalar.dma_start(out=cnt_sb, in_=cnt_d[:])
            bias_sb = const.tile([P, 17], F32)
            nc.scalar.dma_start(out=bias_sb, in_=bias_d[:])
            wlr_sb = const.tile([O, 512], F32)
            nc.scalar.dma_start(out=wlr_sb, in_=wlr_d[:])
            wp_sb = const.tile([P, 8, 1024], BF16)
            for q in range(4):
                nc.scalar.dma_start(
                    out=wp_sb[:, q * 2 : q * 2 + 2, :],
                    in_=wp_d[:, q * 2 : q * 2 + 2, :],
                )
            wa0_sb = const.tile([P, 8, 512], BF16)
            for q in range(2):
                nc.scalar.dma_start(
                    out=wa0_sb[:, q * 4 : q * 4 + 4, :],
                    in_=wa0_d[:, q * 4 : q * 4 + 4, :],
                )
            wa_sb = const.tile([P, 4, 512], BF16)
            nc.scalar.dma_start(out=wa_sb, in_=wa_d[:])
            wl0_sb = const.tile([P, 33, 512], BF16)
            for sl in (slice(0, 12), slice(12, 24), slice(24, 33)):
                nc.scalar.dma_start(out=wl0_sb[:, sl, :], in_=wl0_d[:, sl, :])

            ident = const.tile([P, P], F32)
            make_identity(nc, ident)

            def bcol(i):
                return bias_sb[:, i : i + 1]

            # ---- phase A: segment sums. ps[k, e] = sum over rows of seg k.
            # The two E-halves run concurrently in PE col groups 0 and 1
            # (group from the PSUM slice base partition: 0 -> cols 0-31,
    # 32 -> cols 32-63).
            x_all = act.tile([P, E], F32, tag="x_all")
            for o in range(O):
                htile = htiles[o]
                ps = pseg.tile([P, 512], F32, tag="ps_seg")
                for t in range(T):
                    nc.tensor.matmul(
                        out=ps[0:NK, :],
                        lhsT=mask_sb[:, t, :],
                        rhs=htile[:, t, 0:512],
                        start=(t == 0),
                        stop=(t == T - 1),
                        skip_group_check=True,
                    )
                    nc.tensor.matmul(
                        out=ps[32 : 32 + NK, :],
                        lhsT=mask_sb[:, t, :],
                        rhs=htile[:, t, 512:1024],
                        start=(t == 0),
                        stop=(t == T - 1),
                        skip_group_check=True,
                    )
                nc.vector.tensor_scalar_mul(
                    out=x_all[o * 32 : o * 32 + NK, 0:512],
                    in0=ps[0:NK, :],
                    scalar1=cnt_sb[:, :],
                )
                nc.vector.tensor_scalar_mul(
                    out=x_all[o * 32 : o * 32 + NK, 512:1024],
                    in0=ps[32 : 32 + NK, :],
                    scalar1=cnt_sb[:, :],
                )

            # ---- xT: transpose to feature-major [128, 8, O, NK] bf16
            xTb = act.tile([P, 8, O, 32], BF16)
            for c in range(8):
                ptile = pt.tile([P, P], F32, tag="pt")
                nc.tensor.transpose(
                    out=ptile,
                    in_=x_all[:, c * P : (c + 1) * P],
                    identity=ident[:, :],
                )
                nc.scalar.copy(
                    out=xTb[:, c, :, 0:NK],
                    in_=ptile.rearrange("p (o k) -> p o k", k=32)[:, :, 0:NK],
                )

            # ---- e = max(x @ Wp + (bp+1), 1), classic orientation:
            # Wp [128,128] chunks stationary, xTb streams 52 cols.
            pe_ps = pmm.tile([P, 8, O, NK], F32, tag="pml")
            for m in range(8):
                for k in range(8):
                    nc.tensor.matmul(
                        out=pe_ps[:, m, :, :],
                        lhsT=wp_sb[:, k, m * P : (m + 1) * P],
                        rhs=xTb[:, k, :, 0:NK],
                        start=(k == 0),
                        stop=(k == 7),
                    )
            eTb = act.tile([P, 8, O, NK], BF16)
            eTf = act.tile([P, 8, O, NC10], F32)
            for m in range(8):
                nc.vector.tensor_scalar(
                    out=eTb[:, m, :, :],
                    in0=pe_ps[:, m, :, :],
                    scalar1=bcol(m),
                    scalar2=1.0,
                    op0=OP.add,
                    op1=OP.max,
                )
            for m in range(8):
                nc.vector.tensor_scalar(
                    out=eTf[:, m, :, :],
                    in0=pe_ps[:, m, :, 0:NC10],
                    scalar1=bcol(m),
                    scalar2=1.0,
                    op0=OP.add,
                    op1=OP.max,
                )

            # catF chunks 8..31 (a_ac,b_ac,a_o,b_o,a_q,b_q) only need e.
            catFb = act.tile([P, 33, O], BF16)
            for j, (half, k) in enumerate(
                ((0, 12), (1, 12), (0, 11), (1, 11), (0, 10), (1, 10))
            ):
                nc.gpsimd.tensor_copy(
                    out=catFb[:, 8 + j * 4 : 12 + j * 4, :],
                    in_=eTb[:, half * 4 : half * 4 + 4, :, k],
                )
            # bl0 folding chunk: one-hot stationary row (partition 0 = 1).
            nc.gpsimd.memset(catFb[:, 32, :], 0.0)
            nc.gpsimd.memset(catFb[0:1, 32, :], 1.0)

            def layer(name, w_sb, nk_chunks, rhs_fn, nm, out_free):
                psl = pmm.tile([P, nm] + out_free, F32, tag="pml")
                for m in range(nm):
                    for k in range(nk_chunks):
                        nc.tensor.matmul(
                            out=psl[:, m],
                            lhsT=w_sb[:, k, m * P : (m + 1) * P],
                            rhs=rhs_fn(k),
                            start=(k == 0),
                            stop=(k == nk_chunks - 1),
                        )
                return psl

            # ---- pool 1 (intersection) over the 10 ctx segments
            ph1 = layer("h1", wa0_sb, 8, lambda k: eTb[:, k, :, 0:NC10], 4, [O, NC10])
            h1b = act.tile([P, 4, O, NC10], BF16)
            for m in range(4):
                nc.vector.tensor_scalar(
                    out=h1b[:, m], in0=ph1[:, m], scalar1=bcol(8 + m),
                    scalar2=0.0, op0=OP.add, op1=OP.max,
                )
            pl1 = layer("l1", wa_sb, 4, lambda k: h1b[:, k], 4, [O, NC10])
            l1T = act.tile([P, 4, O, NC10], F32)
            for m in range(4):
                nc.vector.tensor_scalar_add(
                    out=l1T[:, m], in0=pl1[:, m], scalar1=bcol(12 + m)
                )

            # softmax over segments, no max-subtraction (|l1| < ~1.5)
            w1 = act.tile([P, 4, O, NC10], F32)
            nc.scalar.activation(out=w1, in_=l1T, func=AF.Exp)
            s1 = act.tile([P, 4, O], F32)
            nc.vector.reduce_sum(s1, w1, axis=AX)
            r1 = act.tile([P, 4, O], F32)
            nc.vector.reciprocal(out=r1, in_=s1)
            wa_t = act.tile([P, 4, O, NC10], F32)
            nc.vector.tensor_tensor(out=wa_t, in0=w1, in1=eTf[:, 0:4], op=OP.mult)
            suma = tmp.tile([P, 4, O], F32, tag="suma")
            nc.vector.reduce_sum(suma, wa_t, axis=AX)
            cat2 = act.tile([P, 8, O], F32)
            nc.vector.tensor_tensor(out=cat2[:, 0:4, :], in0=suma, in1=r1, op=OP.mult)
            wb_t = act.tile([P, 4, O, NC10], F32)
            nc.gpsimd.tensor_tensor(out=wb_t, in0=w1, in1=eTf[:, 4:8], op=OP.mult)
            sumb = tmp.tile([P, 4, O], F32, tag="sumb")
            nc.vector.reduce_sum(sumb, wb_t, axis=AX)
            nc.gpsimd.tensor_tensor(out=cat2[:, 4:8, :], in0=sumb, in1=r1, op=OP.mult)
            cat2b = act.tile([P, 8, O], BF16)
            nc.vector.tensor_copy(out=cat2b[:, 0:4, :], in_=cat2[:, 0:4, :])
            nc.gpsimd.tensor_copy(out=cat2b[:, 4:8, :], in_=cat2[:, 4:8, :])

            # head partials over catF chunks 8..19 run in this bubble
            pf = phead.tile([P, 512], F32, tag="pf")
            head_started = set()

            def head_mms(kcs, stop_set=()):
                for kc in kcs:
                    g = kc % 2
                    nc.tensor.matmul(
                        out=pf[32 * g : 32 * g + O, :],
                        lhsT=catFb[:, kc, :],
                        rhs=wl0_sb[:, WL0_POS[kc], :],
                        start=(g not in head_started),
                        stop=(kc in stop_set),
                        skip_group_check=True,
                        tile_position=(0, 32 * g),
                    )
                    head_started.add(g)

            head_mms(range(8, 20))

            # ---- renew: h2/l2 on the intersection [O] columns
            ph2 = layer("h2", wa0_sb, 8, lambda k: cat2b[:, k, :], 4, [O])
            h2b = act.tile([P, 4, O], BF16)
            for m in range(4):
                nc.vector.tensor_scalar(
                    out=h2b[:, m], in0=ph2[:, m], scalar1=bcol(8 + m),
                    scalar2=0.0, op0=OP.add, op1=OP.max,
                )
            pl2 = layer("l2", wa_sb, 4, lambda k: h2b[:, k], 4, [O])
            l2T = act.tile([P, 4, O], F32)
            for m in range(4):
                nc.vector.tensor_scalar_add(
                    out=l2T[:, m], in0=pl2[:, m], scalar1=bcol(12 + m)
                )

            # pair softmax([l1, l2]) -> na/nb, store reciprocals.
            # e1 = exp(l1) = w1 (reused), e2 = exp(l2).
            e2 = act.tile([P, 4, O], F32)
            nc.scalar.activation(out=e2, in_=l2T, func=AF.Exp)
            e2b = e2.broadcast_to([P, 4, O, NC10])
            s12 = tmp.tile([P, 4, O, NC10], F32, tag="s12")
            nc.vector.tensor_tensor(out=s12, in0=w1, in1=e2b, op=OP.add)
            rs = tmp.tile([P, 4, O, NC10], F32, tag="rs")
            nc.vector.reciprocal(out=rs, in_=s12)
            raT = act.tile([P, 4, O, NC10], F32)
            rbT = act.tile([P, 4, O, NC10], F32)
            raTb = act.tile([P, 4, O, NC10], BF16)
            rbTb = act.tile([P, 4, O, NC10], BF16)
            for half, wt, cslice, dst, dstb, eng in (
                (0, wa_t, slice(0, 4), raT, raTb, nc.vector),
                (1, wb_t, slice(4, 8), rbT, rbTb, nc.gpsimd),
            ):
                t2 = tmp.tile([P, 4, O], F32, tag=f"t2_{half}")
                eng.tensor_tensor(out=t2, in0=e2, in1=cat2[:, cslice, :], op=OP.mult)
                t3 = tmp.tile([P, 4, O, NC10], F32, tag=f"t3_{half}")
                eng.tensor_tensor(
                    out=t3, in0=wt, in1=t2.broadcast_to([P, 4, O, NC10]), op=OP.add
                )
                nv = tmp.tile([P, 4, O, NC10], F32, tag=f"nv_{half}")
                eng.tensor_tensor(out=nv, in0=t3, in1=rs, op=OP.mult)
                nc.vector.reciprocal(out=dst, in_=nv)
                eng.tensor_copy(out=dstb, in_=dst)

            head_mms(range(20, 32))

            # ---- union pool over segments of [1/na; 1/nb]
            def rhs3(k):
                return raTb[:, k] if k < 4 else rbTb[:, k - 4]

            ph3 = layer("h3", wa0_sb, 8, rhs3, 4, [O, NC10])
            h3b = act.tile([P, 4, O, NC10], BF16)
            for m in range(4):
                nc.vector.tensor_scalar(
                    out=h3b[:, m], in0=ph3[:, m], scalar1=bcol(8 + m),
                    scalar2=0.0, op0=OP.add, op1=OP.max,
                )
            pl3 = layer("l3", wa_sb, 4, lambda k: h3b[:, k], 4, [O, NC10])
            l3T = act.tile([P, 4, O, NC10], F32)
            for m in range(4):
                nc.vector.tensor_scalar_add(
                    out=l3T[:, m], in0=pl3[:, m], scalar1=bcol(12 + m)
                )

            w3 = act.tile([P, 4, O, NC10], F32)
            nc.scalar.activation(out=w3, in_=l3T, func=AF.Exp)
            s3 = act.tile([P, 4, O], F32)
            nc.vector.reduce_sum(s3, w3, axis=AX)
            # ua = s3 / sum(w3 * ra)  (and b on gpsimd)
            for half, src, cslice, eng in (
                (0, raT, slice(0, 4), nc.vector),
                (1, rbT, slice(4, 8), nc.gpsimd),
            ):
                tu = tmp.tile([P, 4, O, NC10], F32, tag=f"tu_{half}")
                eng.tensor_tensor(out=tu, in0=w3, in1=src, op=OP.mult)
                su = tmp.tile([P, 4, O], F32, tag=f"su_{half}")
                nc.vector.reduce_sum(su, tu, axis=AX)
                inv = tmp.tile([P, 4, O], F32, tag=f"inv_{half}")
                nc.vector.reciprocal(out=inv, in_=su)
                eng.tensor_tensor(
                    out=catFb[:, cslice, :], in0=s3, in1=inv, op=OP.mult
                )

            # ---- head tail: union chunks + bl0 chunk. Group 0 (even kc + 32)
            # finishes first so its SBUF copy overlaps group 1's matmuls.
            head_mms([0, 2, 4, 6, 32], stop_set={32})
            c0 = tmp.tile([O, 512], F32, tag="hc0")
            nc.scalar.copy(out=c0, in_=pf[0:O, :])
            head_mms([1, 3, 5, 7], stop_set={7})
            s = tmp.tile([O, 512], F32, tag="hs")
            nc.vector.tensor_tensor(
                out=s, in0=pf[32 : 32 + O, :], in1=c0, op=OP.add
            )
            hw = tmp.tile([O, 512], F32, tag="hw")
            osum = tmp.tile([O, 1], F32, tag="osum")
            nc.vector.scalar_tensor_tensor(
                out=hw, in0=s, scalar=0.0, in1=wlr_sb,
                op0=OP.max, op1=OP.mult, accum_out=osum,
            )
            out_sb = tmp.tile([O, 1], F32, tag="out_sb")
            nc.vector.tensor_scalar_add(
                out=out_sb, in0=osum, scalar1=bias_sb[0:O, 16:17]
            )
            nc.sync.dma_start(out=out_d[:], in_=out_sb)

            if debug:
                for name, t, dt in (
                    ("x_all", x_all, F32),
                    ("eTb", eTb, BF16),
                    ("l1T", l1T, F32),
                    ("cat2", cat2, F32),
                    ("raT", raT, F32),
                    ("rbT", rbT, F32),
                    ("catFb", catFb, BF16),
                    ("hs", s, F32),
                ):
                    d = nc.dram_tensor(
                        "dbg_" + name, list(t.shape), dt, kind="ExternalOutput"
                    )
                    nc.sync.dma_start(out=d[:], in_=t)

    _split_excess_waits(nc)
    return nc


_NC = None


def _get_nc():
    global _NC
    if _NC is None:
        _NC = _build_nc()
    return _NC


def _prep_inputs(hidden, idx, Wp, bp, Wa0, ba0, Wa, ba, Wl0, bl0, Wl, bl):
    hidden = np.asarray(hidden, dtype=np.float32)
    idx = np.asarray(idx).astype(np.int64)

    f32 = lambda a: np.ascontiguousarray(np.asarray(a, dtype=np.float32))
    bf = lambda a: np.ascontiguousarray(
        np.asarray(a, dtype=np.float32).astype(NPBF16)
    )
    bp, ba0, ba, bl0, bl = f32(bp), f32(ba0), f32(ba), f32(bl0), f32(bl)
    Wl = f32(Wl)

    hid8 = np.ascontiguousarray(hidden.astype(NPFP8))  # [B, O, L, E]
    wp_t = bf(np.asarray(Wp, np.float32).reshape(8, P, 1024).transpose(1, 0, 2))
    wa0_t = bf(np.asarray(Wa0, np.float32).reshape(8, P, 512).transpose(1, 0, 2))
    wa_t = bf(np.asarray(Wa, np.float32).reshape(4, P, 512).transpose(1, 0, 2))
    wl0_chunks = np.asarray(Wl0, np.float32).reshape(32, P, 512)
    wl0_t = np.zeros((P, 33, 512), dtype=np.float32)
    for pos, kc in enumerate(WL0_ORDER):
        if kc < 32:
            wl0_t[:, pos, :] = wl0_chunks[kc]
        else:
            wl0_t[0, pos, :] = bl0
    wl0_t = bf(wl0_t)

    biases = np.zeros((P, 17), dtype=np.float32)
    biases[:, 0:8] = (bp + 1.0).reshape(8, P).T
    biases[:, 8:12] = ba0.reshape(4, P).T
    biases[:, 12:16] = ba.reshape(4, P).T
    biases[:, 16] = bl[0]

    wlrep = np.ascontiguousarray(
        np.broadcast_to(Wl[:, 0], (O, 512)).astype(np.float32)
    )

    in_maps = []
    for b in range(B):
        m = np.zeros((L, NK), dtype=np.float32)
        cntinv = np.zeros((NK, 1), dtype=np.float32)
        ib = idx[b]
        starts = [1] + [int(ib[k]) for k in range(9)]
        ends = [int(ib[k]) for k in range(10)]
        segs = [(starts[k], ends[k]) for k in range(10)]
        segs.append((int(ib[9]), int(ib[10])))
        segs.append((int(ib[10]), int(ib[11])))
        segs.append((1, int(ib[9])))
        for k, (s, e) in enumerate(segs):
            m[s:e, k] = 1.0
            cntinv[k, 0] = 1.0 / (e - s)
        maskt = np.ascontiguousarray(
            m.reshape(T, P, NK).transpose(1, 0, 2).astype(NPFP8)
        )

        in_maps.append(
            dict(
                hidden=np.ascontiguousarray(hid8[b]),
                maskt=maskt,
                cntinv=cntinv,
                wp=wp_t,
                wa0=wa0_t,
                wa=wa_t,
                wl0=wl0_t,
                biases=biases,
                wlrep=wlrep,
            )
        )
    return in_maps


def _run(in_maps, **kwargs):
    return run_bass_kernel_spmd(_get_nc(), in_maps, core_ids=list(range(B)), **kwargs)


def kernel(**inputs):
    in_maps = _prep_inputs(**inputs)
    res = _run(in_maps)
    return np.stack([r["out"].reshape(O, 1) for r in res.results])


def _install_ntff_hook():
    """The RL container's antenv lacks axon_hooks, so boot() skipped NTFF
    hook registration. Recreate the module and register the ctypes hook."""
    import sys
    import types

    name = "antenv.axon_hooks"
    if name not in sys.modules:
        try:
            __import__(name)
        except ImportError:
            mod = types.ModuleType(name)
            mod._hook = None
            mod.set_axon_ntff_profile_hook = lambda h: setattr(mod, "_hook", h)
            mod.get_axon_ntff_profile_hook = lambda: mod._hook
            sys.modules[name] = mod
            import antenv

            antenv.axon_hooks = mod
    import antenv.axon_hooks as ah

    if ah.get_axon_ntff_profile_hook() is None:
        from trn_agent_boot.trn_boot import _ntff_profile_via_ctypes

        ah.set_axon_ntff_profile_hook(
            _ntff_profile_via_ctypes("/opt/axon/libaxon_pjrt.so")
        )

    import concourse.bass_utils as bu

    bu.upload_artifacts = lambda tmpdir: tmpdir


def benchmark(trace_cores=None, **inputs):
    """Run with NTFF tracing; returns (output, BassKernelResults)."""
    _install_ntff_hook()
    in_maps = _prep_inputs(**inputs)
    res = _run(in_maps, trace=True, trace_cores=trace_cores)
    out = np.stack([r["out"].reshape(O, 1) for r in res.results])
    return out, res


# revision 24
# speedup vs baseline: 1.6129x; 1.1083x over previous
"""Trainium2 Bass kernel for nn_Beta_LR_41308995453190.

Network (per (b, o) pair):
  - 13 segment means over the L axis of hidden[b, o] (ragged boundaries
    from idx[b]): 10 context segments, question, option, whole-context.
  - beta-param projection e = 1 + relu(x @ Wp + bp), split a/b.
  - three attention pools (intersection over segments, renew over
    (segment, intersection) pairs, union over inverted renewed params).
  - classify head: concat 8 beta embeddings -> relu(@Wl0 + bl0) -> @Wl + bl.

Sharding: data-parallel over the batch dim B=8 (one batch per NeuronCore),
weights replicated.

Design (v2 — rebuilt around the trace of the v1 kernel):
  - hidden travels in fp8 e3m4 (4.2 MB/core instead of 8.4 bf16; measured
    end-to-end rel-err 2.9e-3 vs the 2e-2 gate). All weights bf16.
  - Segment sums are 0/1-mask matmuls (mask stationary, hidden streaming).
    The two E-halves run CONCURRENTLY in separate PE column groups
    (tile_position col 0 / 32, derived from the PSUM slice base partition).
  - The beta-network layers run in CLASSIC orientation: weight chunks
    [128, 128] stationary, feature-major activations [128, cols] streaming.
    Layer outputs land feature-major in PSUM, so the bias/relu DVE op is
    128-partition-parallel and NO transposes are needed between layers
    (v1 spent ~10us of PE time on 40 transposes + PSUM round trips).
  - Softmaxes skip the max-subtraction (logits are ~N(0, 0.25)); the
    intersection's exp/weighted sums are reused by the renew stage.
  - Classify head: catF chunks stationary [128, 4], wl0 streams 512 wide,
    accumulated in 4 PE column groups concurrently; bl0 is folded in as a
    33rd contraction chunk (one-hot stationary, bl0 in wl0 row 0). The 24
    chunks that only need the projection run inside the softmax bubbles;
    epilogue relu*Wl+reduce is one fused scalar_tensor_tensor op.
  - DMA: hidden kicks on the Sync HWDGE queue, weights on the Scalar
    queue (two engines issue descriptors concurrently; each descriptor
    fans out to one of 16 HW DMA engines). wl0 (4.2 MB) is ordered last
    — the head only needs it ~25us in.
"""

import numpy as np
import ml_dtypes

try:
    import concourse.bass as bass
except ImportError:
    import sys

    sys.path.insert(0, "/opt/trn_rl_repo")
    import concourse.bass as bass

import concourse.tile as tile
from concourse import mybir
from concourse.bass_utils import run_bass_kernel_spmd
from concourse.masks import make_identity

F32 = mybir.dt.float32
BF16 = mybir.dt.bfloat16
FP8 = mybir.dt.float8e3  # e3m4
NPBF16 = ml_dtypes.bfloat16
NPFP8 = ml_dtypes.float8_e3m4
AX = mybir.AxisListType.X
OP = mybir.AluOpType
AF = mybir.ActivationFunctionType

B, O, L, E = 8, 4, 1024, 1024
BETA = 512
NSEG = 12
NK = 13  # 10 ctx + q + o + allc
NC10 = 10  # pooled ctx segments
P = 128
T = L // P  # 8 L-tiles per option
NCOL = O * NK  # 52

# wl0 host chunk order: natural (the head runs at the very end, after wl0
# has fully arrived; issuing it earlier would stall the in-order PE queue).
WL0_ORDER = list(range(33))
WL0_POS = {kc: i for i, kc in enumerate(WL0_ORDER)}


def _patch_minimal_drain():
    """One-shot NEFF: skip the semaphore-clear + second all-engine barrier of
    the TileContext epilogue (they only matter when the program loops).
    Worth ~6us of measured exec time (the ~150 per-engine semaphore clears
    serialize at 20-50ns each)."""
    from concourse.vector_clock import ScopedClock

    def _drain_and_barrier(self, tick_clock, wait_clock):
        drain_inst = self.nc.sync.drain()
        wait_clock.add_sem_waits(
            drain_inst.ins, ScopedClock({None: tick_clock.global_clock})
        )
        self.nc.all_engine_barrier()
        assert self.sems is not None
        popped = self.nc._tile_sem_poison_stack.pop()
        assert popped is self._sem_poison
    tile.TileContext._drain_and_barrier = _drain_and_barrier


_patch_minimal_drain()


def _split_excess_waits(nc, max_waits=1):
    """This neuronxcc walrus build rejects more than one sem wait per TPB
    instruction ("Too many sync wait commands"). Hoist excess waits onto
    drain instructions inserted immediately before the offending instruction
    on the same engine."""
    scratch_bb = nc.cur_bb.bb
    for f in nc.m.functions:
        for bb in f.blocks:
            new_list = []
            for ins in bb.instructions:
                si = ins.sync_info
                waits = list(si.on_wait) if si and si.on_wait else []
                if len(waits) > max_waits:
                    for w in waits[: len(waits) - max_waits]:
                        carrier = nc.engines[ins.engine].nop(nofuse=True).ins
                        scratch_bb.instructions.remove(carrier)
                        carrier.sync_info = mybir.SyncInfo(
                            on_wait=[w], on_update=[]
                        )
                        new_list.append(carrier)
                    si.on_wait = waits[len(waits) - max_waits :]
                new_list.append(ins)
            bb.instructions[:] = new_list


def _build_nc(debug=False):
    nc = bass.Bass("TRN2", target_bir_lowering=False)

    hid_d = nc.dram_tensor("hidden", [O, L, E], FP8, kind="ExternalInput")
    mask_d = nc.dram_tensor("maskt", [P, T, NK], FP8, kind="ExternalInput")
    cnt_d = nc.dram_tensor("cntinv", [NK, 1], F32, kind="ExternalInput")
    wp_d = nc.dram_tensor("wp", [P, 8, 1024], BF16, kind="ExternalInput")
    wa0_d = nc.dram_tensor("wa0", [P, 8, 512], BF16, kind="ExternalInput")
    wa_d = nc.dram_tensor("wa", [P, 4, 512], BF16, kind="ExternalInput")
    wl0_d = nc.dram_tensor("wl0", [P, 33, 512], BF16, kind="ExternalInput")
    bias_d = nc.dram_tensor("biases", [P, 17], F32, kind="ExternalInput")
    wlr_d = nc.dram_tensor("wlrep", [O, 512], F32, kind="ExternalInput")
    out_d = nc.dram_tensor("out", [O, 1], F32, kind="ExternalOutput")

    with tile.TileContext(nc) as tc:
        with (
            tc.tile_pool(name="const", bufs=1) as const,
            tc.tile_pool(name="hidp2", bufs=2) as hidp2,
            tc.tile_pool(name="act", bufs=1) as act,
            tc.tile_pool(name="tmp", bufs=2) as tmp,
            tc.tile_pool(name="pseg", bufs=2, space="PSUM") as pseg,
            tc.tile_pool(name="pt", bufs=2, space="PSUM") as pt,
            tc.tile_pool(name="pmm", bufs=2, space="PSUM") as pmm,
            tc.tile_pool(name="phead", bufs=1, space="PSUM") as phead,
        ):
            # ---- DMA kicks. Two HWDGE queues (Sync, Scalar) issue
            # descriptors concurrently; each descriptor fans out to one of
            # 16 HW DMA engines (~24.5 GB/s per engine, ~358 GB/s aggregate,
            # FIFO per engine). Hidden options are interleaved across both
            # queues so all 16 hidden chunks land on distinct engines first;
            # weights follow (wp/wa0 split by output block so the consuming
            # layers pipeline); wl0 is last (head needs it at ~40us).
            mask_sb = const.tile([P, T, NK], FP8)
            nc.sync.dma_start(out=mask_sb, in_=mask_d[:])
            cnt_sb = const.tile([NK, 1], F32)
            nc.scalar.dma_start(out=cnt_sb, in_=cnt_d[:])
            bias_sb = const.tile([P, 17], F32)
            nc.scalar.dma_start(out=bias_sb, in_=bias_d[:])
            wlr_sb = const.tile([O, 512], F32)
            nc.scalar.dma_start(out=wlr_sb, in_=wlr_d[:])

            hid_r = hid_d.rearrange("o (t p) e -> o p t e", p=P)
            htiles = [
                hidp2.tile([P, T, E], FP8, tag="htile", name=f"htile{o}")
                for o in range(O)
            ]
            for o in range(O):
                eng = nc.sync if o % 2 == 0 else nc.scalar
                for q in range(4):
                    eng.dma_start(
                        out=htiles[o][:, q * 2 : q * 2 + 2, :],
                        in_=hid_r[o][:, q * 2 : q * 2 + 2, :],
                    )

            wp_sb = const.tile([P, 8, 1024], BF16)
            for j in range(8):
                nc.sync.dma_start(
                    out=wp_sb[:, j : j + 1, :], in_=wp_d[:, j : j + 1, :]
                )
            wa0_sb = const.tile([P, 8, 512], BF16)
            for j in range(4):
                nc.scalar.dma_start(
                    out=wa0_sb[:, j * 2 : j * 2 + 2, :],
                    in_=wa0_d[:, j * 2 : j * 2 + 2, :],
                )
            wa_sb = const.tile([P, 4, 512], BF16)
            for j in range(2):
                nc.scalar.dma_start(
                    out=wa_sb[:, j * 2 : j * 2 + 2, :],
                    in_=wa_d[:, j * 2 : j * 2 + 2, :],
                )
            wl0_sb = const.tile([P, 33, 512], BF16)
            for j in range(11):
                nc.sync.dma_start(
                    out=wl0_sb[:, j * 3 : j * 3 + 3, :],
                    in_=wl0_d[:, j * 3 : j * 3 + 3, :],
                )

            ident = const.tile([P, P], F32)
            make_identity(nc, ident)

            def bcol(i):
                return bias_sb[:, i : i + 1]

            # ---- phase A: segment sums. ps[k, e] = sum over rows of seg k.
            # The two E-halves run concurrently in PE col groups 0 and 1
            # (group from the PSUM slice base partition: 0 -> cols 0-31,
    # 32 -> cols 32-63).
            x_all = act.tile([P, E], F32, tag="x_all")
            for o in range(O):
                htile = htiles[o]
                ps = pseg.tile([P, 512], F32, tag="ps_seg")
                for t in range(T):
                    nc.tensor.matmul(
                        out=ps[0:NK, :],
                        lhsT=mask_sb[:, t, :],
                        rhs=htile[:, t, 0:512],
                        start=(t == 0),
                        stop=(t == T - 1),
                        skip_group_check=True,
                    )
                    nc.tensor.matmul(
                        out=ps[32 : 32 + NK, :],
                        lhsT=mask_sb[:, t, :],
                        rhs=htile[:, t, 512:1024],
                        start=(t == 0),
                        stop=(t == T - 1),
                        skip_group_check=True,
                    )
                nc.vector.tensor_scalar_mul(
                    out=x_all[o * 32 : o * 32 + NK, 0:512],
                    in0=ps[0:NK, :],
                    scalar1=cnt_sb[:, :],
                )
                nc.vector.tensor_scalar_mul(
                    out=x_all[o * 32 : o * 32 + NK, 512:1024],
                    in0=ps[32 : 32 + NK, :],
                    scalar1=cnt_sb[:, :],
                )

            # ---- xT: transpose to feature-major [128, 8, O, NK] bf16
            xTb = act.tile([P, 8, O, 32], BF16)
            for c in range(8):
                ptile = pt.tile([P, P], F32, tag="pt")
                nc.tensor.transpose(
                    out=ptile,
                    in_=x_all[:, c * P : (c + 1) * P],
                    identity=ident[:, :],
                )
                nc.scalar.copy(
                    out=xTb[:, c, :, 0:NK],
                    in_=ptile.rearrange("p (o k) -> p o k", k=32)[:, :, 0:NK],
                )

            # ---- e = max(x @ Wp + (bp+1), 1), classic orientation:
            # Wp [128,128] chunks stationary, xTb streams 52 cols. k-outer
            # so the matmuls start as each wp k-chunk lands from DMA.
            # NOTE: chains must run one-at-a-time (m-outer): a matmul with
            # start=True clears its full partition range in the target bank,
            # so interleaving same-partition accumulation chains loses the
            # earlier chains' first terms (verified on HW).
            pe_ps = pmm.tile([P, 8, O, NK], F32, tag="pml")
            for m in range(8):
                for k in range(8):
                    nc.tensor.matmul(
                        out=pe_ps[:, m, :, :],
                        lhsT=wp_sb[:, k, m * P : (m + 1) * P],
                        rhs=xTb[:, k, :, 0:NK],
                        start=(k == 0),
                        stop=(k == 7),
                    )
            eTb = act.tile([P, 8, O, NK], BF16)
            eTf = act.tile([P, 8, O, NC10], F32)
            for m in range(8):
                nc.vector.tensor_scalar(
                    out=eTb[:, m, :, :],
                    in0=pe_ps[:, m, :, :],
                    scalar1=bcol(m),
                    scalar2=1.0,
                    op0=OP.add,
                    op1=OP.max,
                )
            # fp32 copy for the pool arithmetic — off the critical path,
            # gpsimd can't read PSUM so it reads back through eTb... instead
            # keep it on vector but AFTER eTb (h1 only gates on eTb).
            for m in range(8):
                nc.vector.tensor_scalar(
                    out=eTf[:, m, :, :],
                    in0=pe_ps[:, m, :, 0:NC10],
                    scalar1=bcol(m),
                    scalar2=1.0,
                    op0=OP.add,
                    op1=OP.max,
                )

            # catF chunks 8..31 (a_ac,b_ac,a_o,b_o,a_q,b_q) only need e.
            catFb = act.tile([P, 33, O], BF16)
            for j, (half, k) in enumerate(
                ((0, 12), (1, 12), (0, 11), (1, 11), (0, 10), (1, 10))
            ):
                nc.gpsimd.tensor_copy(
                    out=catFb[:, 8 + j * 4 : 12 + j * 4, :],
                    in_=eTb[:, half * 4 : half * 4 + 4, :, k],
                )
            # bl0 folding chunk: one-hot stationary row (partition 0 = 1).
            nc.gpsimd.memset(catFb[:, 32, :], 0.0)
            nc.gpsimd.memset(catFb[0:1, 32, :], 1.0)

            def layer(name, w_sb, nk_chunks, rhs_fn, nm, out_free):
                psl = pmm.tile(
                    [P, nm] + out_free, F32, tag="pml", name=f"ps_{name}"
                )
                for m in range(nm):
                    for k in range(nk_chunks):
                        nc.tensor.matmul(
                            out=psl[:, m],
                            lhsT=w_sb[:, k, m * P : (m + 1) * P],
                            rhs=rhs_fn(k),
                            start=(k == 0),
                            stop=(k == nk_chunks - 1),
                        )
                return psl

            # ---- pool 1 (intersection) over the 10 ctx segments
            ph1 = layer("h1", wa0_sb, 8, lambda k: eTb[:, k, :, 0:NC10], 4, [O, NC10])
            h1b = act.tile([P, 4, O, NC10], BF16)
            for m in range(4):
                nc.vector.tensor_scalar(
                    out=h1b[:, m], in0=ph1[:, m], scalar1=bcol(8 + m),
                    scalar2=0.0, op0=OP.add, op1=OP.max,
                )
            pl1 = layer("l1", wa_sb, 4, lambda k: h1b[:, k], 4, [O, NC10])
            l1T = act.tile([P, 4, O, NC10], F32)
            for m in range(4):
                nc.vector.tensor_scalar_add(
                    out=l1T[:, m], in0=pl1[:, m], scalar1=bcol(12 + m)
                )

            # softmax over segments, no max-subtraction (|l1| < ~1.5).
            # Shared factors (w1, r1, ...) are duplicated across the a/b
            # chunk halves by gpsimd copies so each chain step is ONE wide
            # vector op instead of two serialized halves.
            w1x2 = act.tile([P, 8, O, NC10], F32)
            w1 = w1x2[:, 0:4]
            nc.scalar.activation(out=w1, in_=l1T, func=AF.Exp)
            nc.gpsimd.tensor_copy(out=w1x2[:, 4:8], in_=w1)
            s1 = act.tile([P, 4, O], F32)
            nc.vector.reduce_sum(s1, w1, axis=AX)
            r1x2 = act.tile([P, 8, O], F32)
            nc.vector.reciprocal(out=r1x2[:, 0:4, :], in_=s1)
            nc.gpsimd.tensor_copy(out=r1x2[:, 4:8, :], in_=r1x2[:, 0:4, :])
            wab_t = act.tile([P, 8, O, NC10], F32)
            nc.vector.tensor_tensor(out=wab_t, in0=w1x2, in1=eTf, op=OP.mult)
            sumab = tmp.tile([P, 8, O], F32, tag="sumab")
            nc.vector.reduce_sum(sumab, wab_t, axis=AX)
            cat2 = act.tile([P, 8, O], F32)
            nc.vector.tensor_tensor(out=cat2, in0=sumab, in1=r1x2, op=OP.mult)
            cat2b = act.tile([P, 8, O], BF16)
            nc.vector.tensor_copy(out=cat2b, in_=cat2)

            # ---- renew: h2/l2 on the intersection [O] columns
            ph2 = layer("h2", wa0_sb, 8, lambda k: cat2b[:, k, :], 4, [O])
            h2b = act.tile([P, 4, O], BF16)
            for m in range(4):
                nc.vector.tensor_scalar(
                    out=h2b[:, m], in0=ph2[:, m], scalar1=bcol(8 + m),
                    scalar2=0.0, op0=OP.add, op1=OP.max,
                )
            pl2 = layer("l2", wa_sb, 4, lambda k: h2b[:, k], 4, [O])
            l2T = act.tile([P, 4, O], F32)
            for m in range(4):
                nc.vector.tensor_scalar_add(
                    out=l2T[:, m], in0=pl2[:, m], scalar1=bcol(12 + m)
                )

            # pair softmax([l1, l2]) -> na/nb, store reciprocals.
            # e1 = exp(l1) = w1 (reused), e2 = exp(l2).
            e2x2 = act.tile([P, 8, O], F32)
            e2 = e2x2[:, 0:4, :]
            nc.scalar.activation(out=e2, in_=l2T, func=AF.Exp)
            nc.gpsimd.tensor_copy(out=e2x2[:, 4:8, :], in_=e2)
            s12 = tmp.tile([P, 4, O, NC10], F32, tag="s12")
            nc.vector.tensor_tensor(
                out=s12, in0=w1, in1=e2.broadcast_to([P, 4, O, NC10]), op=OP.add
            )
            rsx2 = act.tile([P, 8, O, NC10], F32)
            nc.vector.reciprocal(out=rsx2[:, 0:4], in_=s12)
            nc.gpsimd.tensor_copy(out=rsx2[:, 4:8], in_=rsx2[:, 0:4])
            t2ab = tmp.tile([P, 8, O], F32, tag="t2ab")
            nc.vector.tensor_tensor(out=t2ab, in0=e2x2, in1=cat2, op=OP.mult)
            t3ab = tmp.tile([P, 8, O, NC10], F32, tag="t3ab")
            nc.vector.tensor_tensor(
                out=t3ab, in0=wab_t,
                in1=t2ab.broadcast_to([P, 8, O, NC10]), op=OP.add,
            )
            nvab = tmp.tile([P, 8, O, NC10], F32, tag="nvab")
            nc.vector.tensor_tensor(out=nvab, in0=t3ab, in1=rsx2, op=OP.mult)
            rab = act.tile([P, 8, O, NC10], F32)
            nc.vector.reciprocal(out=rab, in_=nvab)
            rabb = act.tile([P, 8, O, NC10], BF16)
            nc.vector.tensor_copy(out=rabb, in_=rab)

            # ---- union pool over segments of [1/na; 1/nb]
            ph3 = layer("h3", wa0_sb, 8, lambda k: rabb[:, k], 4, [O, NC10])
            h3b = act.tile([P, 4, O, NC10], BF16)
            for m in range(4):
                nc.vector.tensor_scalar(
                    out=h3b[:, m], in0=ph3[:, m], scalar1=bcol(8 + m),
                    scalar2=0.0, op0=OP.add, op1=OP.max,
                )
            pl3 = layer("l3", wa_sb, 4, lambda k: h3b[:, k], 4, [O, NC10])
            l3T = act.tile([P, 4, O, NC10], F32)
            for m in range(4):
                nc.vector.tensor_scalar_add(
                    out=l3T[:, m], in0=pl3[:, m], scalar1=bcol(12 + m)
                )

            w3x2 = act.tile([P, 8, O, NC10], F32)
            w3 = w3x2[:, 0:4]
            nc.scalar.activation(out=w3, in_=l3T, func=AF.Exp)
            nc.gpsimd.tensor_copy(out=w3x2[:, 4:8], in_=w3)
            s3x2 = act.tile([P, 8, O], F32)
            nc.vector.reduce_sum(s3x2[:, 0:4, :], w3, axis=AX)
            nc.gpsimd.tensor_copy(out=s3x2[:, 4:8, :], in_=s3x2[:, 0:4, :])
            # ua = s3 / sum(w3 * ra) ; ub likewise, one wide op per step
            tuab = tmp.tile([P, 8, O, NC10], F32, tag="tuab")
            nc.vector.tensor_tensor(out=tuab, in0=w3x2, in1=rab, op=OP.mult)
            suab = tmp.tile([P, 8, O], F32, tag="suab")
            nc.vector.reduce_sum(suab, tuab, axis=AX)
            invab = tmp.tile([P, 8, O], F32, tag="invab")
            nc.vector.reciprocal(out=invab, in_=suab)
            nc.vector.tensor_tensor(
                out=catFb[:, 0:8, :], in0=s3x2, in1=invab, op=OP.mult
            )

            # ---- head: all 33 chunks, 2 PE col groups. wl0 has been on-chip
            # since ~40us. Group 0 (even kc + 32) finishes first so its SBUF
            # copy overlaps group 1's matmuls.
            pf = phead.tile([P, 512], F32, tag="pf")
            head_started = set()

            def head_mms(kcs, stop_set=()):
                for kc in kcs:
                    g = kc % 2
                    nc.tensor.matmul(
                        out=pf[32 * g : 32 * g + O, :],
                        lhsT=catFb[:, kc, :],
                        rhs=wl0_sb[:, WL0_POS[kc], :],
                        start=(g not in head_started),
                        stop=(kc in stop_set),
                        skip_group_check=True,
                        tile_position=(0, 32 * g),
                    )
                    head_started.add(g)

            head_mms([kc for kc in range(8, 32)])
            head_mms([0, 2, 4, 6, 32], stop_set={32})
            c0 = tmp.tile([O, 512], F32, tag="hc0")
            nc.scalar.copy(out=c0, in_=pf[0:O, :])
            head_mms([1, 3, 5, 7], stop_set={7})
            s = tmp.tile([O, 512], F32, tag="hs")
            nc.vector.tensor_tensor(
                out=s, in0=pf[32 : 32 + O, :], in1=c0, op=OP.add
            )
            hw = tmp.tile([O, 512], F32, tag="hw")
            osum = tmp.tile([O, 1], F32, tag="osum")
            nc.vector.scalar_tensor_tensor(
                out=hw, in0=s, scalar=0.0, in1=wlr_sb,
                op0=OP.max, op1=OP.mult, accum_out=osum,
            )
            out_sb = tmp.tile([O, 1], F32, tag="out_sb")
            nc.vector.tensor_scalar_add(
                out=out_sb, in0=osum, scalar1=bias_sb[0:O, 16:17]
            )
            nc.sync.dma_start(out=out_d[:], in_=out_sb)

            if debug:
                for name, t, dt in (
                    ("x_all", x_all, F32),
                    ("eTb", eTb, BF16),
                    ("l1T", l1T, F32),
                    ("cat2", cat2, F32),
                    ("rab", rab, F32),
                    ("catFb", catFb, BF16),
                    ("hs", s, F32),
                ):
                    d = nc.dram_tensor(
                        "dbg_" + name, list(t.shape), dt, kind="ExternalOutput"
                    )
                    nc.sync.dma_start(out=d[:], in_=t)

    _split_excess_waits(nc)
    return nc


_NC = None


def _get_nc():
    global _NC
    if _NC is None:
        _NC = _build_nc()
    return _NC


def _prep_inputs(hidden, idx, Wp, bp, Wa0, ba0, Wa, ba, Wl0, bl0, Wl, bl):
    hidden = np.asarray(hidden, dtype=np.float32)
    idx = np.asarray(idx).astype(np.int64)

    f32 = lambda a: np.ascontiguousarray(np.asarray(a, dtype=np.float32))
    bf = lambda a: np.ascontiguousarray(
        np.asarray(a, dtype=np.float32).astype(NPBF16)
    )
    bp, ba0, ba, bl0, bl = f32(bp), f32(ba0), f32(ba), f32(bl0), f32(bl)
    Wl = f32(Wl)

    hid8 = np.ascontiguousarray(hidden.astype(NPFP8))  # [B, O, L, E]
    wp_t = bf(np.asarray(Wp, np.float32).reshape(8, P, 1024).transpose(1, 0, 2))
    wa0_t = bf(np.asarray(Wa0, np.float32).reshape(8, P, 512).transpose(1, 0, 2))
    wa_t = bf(np.asarray(Wa, np.float32).reshape(4, P, 512).transpose(1, 0, 2))
    wl0_chunks = np.asarray(Wl0, np.float32).reshape(32, P, 512)
    wl0_t = np.zeros((P, 33, 512), dtype=np.float32)
    for pos, kc in enumerate(WL0_ORDER):
        if kc < 32:
            wl0_t[:, pos, :] = wl0_chunks[kc]
        else:
            wl0_t[0, pos, :] = bl0
    wl0_t = bf(wl0_t)

    biases = np.zeros((P, 17), dtype=np.float32)
    biases[:, 0:8] = (bp + 1.0).reshape(8, P).T
    biases[:, 8:12] = ba0.reshape(4, P).T
    biases[:, 12:16] = ba.reshape(4, P).T
    biases[:, 16] = bl[0]

    wlrep = np.ascontiguousarray(
        np.broadcast_to(Wl[:, 0], (O, 512)).astype(np.float32)
    )

    in_maps = []
    for b in range(B):
        m = np.zeros((L, NK), dtype=np.float32)
        cntinv = np.zeros((NK, 1), dtype=np.float32)
        ib = idx[b]
        starts = [1] + [int(ib[k]) for k in range(9)]
        ends = [int(ib[k]) for k in range(10)]
        segs = [(starts[k], ends[k]) for k in range(10)]
        segs.append((int(ib[9]), int(ib[10])))
        segs.append((int(ib[10]), int(ib[11])))
        segs.append((1, int(ib[9])))
        for k, (s, e) in enumerate(segs):
            m[s:e, k] = 1.0
            cntinv[k, 0] = 1.0 / (e - s)
        maskt = np.ascontiguousarray(
            m.reshape(T, P, NK).transpose(1, 0, 2).astype(NPFP8)
        )

        in_maps.append(
            dict(
                hidden=np.ascontiguousarray(hid8[b]),
                maskt=maskt,
                cntinv=cntinv,
                wp=wp_t,
                wa0=wa0_t,
                wa=wa_t,
                wl0=wl0_t,
                biases=biases,
                wlrep=wlrep,
            )
        )
    return in_maps


def _run(in_maps, **kwargs):
    return run_bass_kernel_spmd(_get_nc(), in_maps, core_ids=list(range(B)), **kwargs)


def kernel(**inputs):
    in_maps = _prep_inputs(**inputs)
    res = _run(in_maps)
    return np.stack([r["out"].reshape(O, 1) for r in res.results])


def _install_ntff_hook():
    """The RL container's antenv lacks axon_hooks, so boot() skipped NTFF
    hook registration. Recreate the module and register the ctypes hook."""
    import sys
    import types

    name = "antenv.axon_hooks"
    if name not in sys.modules:
        try:
            __import__(name)
        except ImportError:
            mod = types.ModuleType(name)
            mod._hook = None
            mod.set_axon_ntff_profile_hook = lambda h: setattr(mod, "_hook", h)
            mod.get_axon_ntff_profile_hook = lambda: mod._hook
            sys.modules[name] = mod
            import antenv

            antenv.axon_hooks = mod
    import antenv.axon_hooks as ah

    if ah.get_axon_ntff_profile_hook() is None:
        from trn_agent_boot.trn_boot import _ntff_profile_via_ctypes

        ah.set_axon_ntff_profile_hook(
            _ntff_profile_via_ctypes("/opt/axon/libaxon_pjrt.so")
        )

    import concourse.bass_utils as bu

    bu.upload_artifacts = lambda tmpdir: tmpdir


def benchmark(trace_cores=None, **inputs):
    """Run with NTFF tracing; returns (output, BassKernelResults)."""
    _install_ntff_hook()
    in_maps = _prep_inputs(**inputs)
    res = _run(in_maps, trace=True, trace_cores=trace_cores)
    out = np.stack([r["out"].reshape(O, 1) for r in res.results])
    return out, res
